# revision 5
# baseline (speedup 1.0000x reference)
"""Trainium2 Bass kernel for 2-layer GCN (nn_GCN_22866405884174).

Strategy (8 NeuronCores, dst-node sharding):
  out = A @ relu((A @ x) @ W1 + b1) @ W2 + b2   with A = D^-1/2 (Adj+I) D^-1/2
  (linear layers commute with aggregation, so each layer is: gather table
  rows by edge src + scatter-add by edge dst, then a small dense matmul).

  - Nodes are sharded contiguously: core c owns dst nodes [c*12500, (c+1)*12500).
  - Graph preprocessing on host (degrees, D^-1/2, edge sort, one-hot scatter
    blocks): pure index/graph-partitioning work per the sharding hint.
  - Layer tables are bf16, pre-scaled by dinv[src]; dinv[dst] is applied
    post-aggregation on-device. Scatter matrices S are exact one-hot fp8.
  - Gather: SWDGE dma_gather (256B rows) into SBUF message tiles.
  - Scatter-add: PE matmul  msg[slots,feat]^T @ S[slots,dst]  accumulated in
    PSUM per 64-dst window -> feature-major agg in SBUF.
  - Between layers: AllGather of the bf16 hidden table across the 8 cores.
"""

import numpy as np
import ml_dtypes

# ---------------- problem constants (hardcoded per contract) ----------------
N = 100000
E = 1600000
F_IN = 128
HID = 64
OUT_D = 10

NCORES = 8
NPC = N // NCORES           # 12500 nodes per core
SH = 12544                  # padded shard rows (98 * 128)
NTOT = SH * NCORES          # 100352
SEC = 25088                 # table section rows (2 shards, < int16 range)
NSEC = 4
WDST = 64                   # dst window width
NWIN = (NPC + WDST - 1) // WDST   # 196
NG = SH // 128              # 98 node groups per shard
SENTINEL = 12500            # zero pad row (local within shard -> same local idx per section)

_CACHE = {}


# ============================ host preprocessing ============================

def _host_prep(edge_index):
    """Graph preprocessing: degrees, edge sort, per-core gather/scatter plans."""
    src = np.asarray(edge_index[0]).astype(np.int64)
    dst = np.asarray(edge_index[1]).astype(np.int64)
    # self loops
    loops = np.arange(N, dtype=np.int64)
    src = np.concatenate([src, loops])
    dst = np.concatenate([dst, loops])
    deg = np.bincount(dst, minlength=N).astype(np.float32)  # includes self loop
    dinv = 1.0 / np.sqrt(deg)

    srow = (src // NPC) * SH + (src % NPC)      # shard-layout table row of src
    core = dst // NPC
    dloc = dst % NPC
    win = dloc // WDST
    sec = srow // SEC

    # per (core, win, sec) counts -> shared schedule = max over cores
    cellid = (core * NWIN + win) * NSEC + sec
    counts = np.bincount(cellid, minlength=NCORES * NWIN * NSEC).reshape(NCORES, NWIN, NSEC)
    n_cell = counts.max(axis=0)                 # [NWIN, NSEC]
    n_cell = np.maximum(n_cell, 16)

    # schedule offsets (python ints, identical across cores)
    idx_cols = []       # per cell: number of int16 columns
    grp_cnt = []        # per cell: number of 128-slot groups
    for w in range(NWIN):
        for s in range(NSEC):
            n = int(n_cell[w, s])
            idx_cols.append((n + 15) // 16)
            grp_cnt.append((n + 127) // 128)
    CIDX = int(np.sum(idx_cols))
    TG = int(np.sum(grp_cnt))

    # per-core sorted edge arrays
    order = np.lexsort((sec + 4 * win, core))   # by core, then window, then section
    srow_s = srow[order]
    dloc_s = dloc[order]
    core_s = core[order]
    sec_s = sec[order]
    win_s = win[order]

    idx_all = np.zeros((NCORES, 128, CIDX), dtype=np.int16)
    sval_all = np.zeros((NCORES, TG, 128, WDST), dtype=ml_dtypes.float8_e4m3)
    dinv_gt = np.zeros((NCORES, 128, NG), dtype=np.float32)

    core_starts = np.searchsorted(core_s, np.arange(NCORES + 1))
    for c in range(NCORES):
        lo, hi = core_starts[c], core_starts[c + 1]
        sr = srow_s[lo:hi]
        dl = dloc_s[lo:hi]
        wv = win_s[lo:hi]
        sv = sec_s[lo:hi]
        key = wv * NSEC + sv
        cell_starts = np.searchsorted(key, np.arange(NWIN * NSEC + 1))

        # build padded local-idx + slot->dst mapping per cell, vectorized-ish
        ci = 0   # idx col offset
        gi = 0   # group offset
        # accumulate into flat S via coordinate lists
        s_g = []
        s_p = []
        s_d = []
        for w in range(NWIN):
            for s in range(NSEC):
                cid = w * NSEC + s
                a, b = cell_starts[cid], cell_starts[cid + 1]
                n = int(n_cell[w, s])
                cnt = b - a
                loc = np.full(n, SENTINEL, dtype=np.int64)
                loc[:cnt] = sr[a:b] - s * SEC
                ncol = (n + 15) // 16
                pad16 = np.zeros(ncol * 16, dtype=np.int16)
                pad16[:n] = loc.astype(np.int16)
                idx_all[c, :, ci:ci + ncol] = np.tile(pad16.reshape(ncol, 16).T, (8, 1))
                # S one-hot for real slots only
                t = np.arange(cnt)
                s_g.append(gi + t // 128)
                s_p.append(t % 128)
                s_d.append(dl[a:b] - w * WDST)
                ci += ncol
                gi += (n + 127) // 128
        s_g = np.concatenate(s_g)
        s_p = np.concatenate(s_p)
        s_d = np.concatenate(s_d)
        sval_all[c, s_g, s_p, s_d] = ml_dtypes.float8_e4m3(1.0)

        dvc = dinv[c * NPC:(c + 1) * NPC]
        dpad = np.zeros(SH, dtype=np.float32)
        dpad[:NPC] = dvc
        dinv_gt[c] = dpad.reshape(NG, 128).T

    sched = {
        "n_cell": [[int(n_cell[w, s]) for s in range(NSEC)] for w in range(NWIN)],
        "CIDX": CIDX,
        "TG": TG,
    }
    return sched, idx_all, sval_all, dinv_gt, dinv


# ============================ device program ============================

def build_program(sched):
    import concourse.bass as bass
    import concourse.bacc as bacc
    import concourse.tile as tile
    import concourse.mybir as mybir

    CIDX = sched["CIDX"]
    TG = sched["TG"]
    n_cell = sched["n_cell"]
    GMAX = max((n + 127) // 128 for row in n_cell for n in row)

    nc = bacc.Bacc(None, target_bir_lowering=False, debug=False)
    f32 = mybir.dt.float32
    bf16 = mybir.dt.bfloat16
    fp8 = mybir.dt.float8e4
    i16 = mybir.dt.int16

    T1 = nc.dram_tensor("T1", [NTOT, F_IN], bf16, kind="ExternalInput")
    IDX = nc.dram_tensor("IDX", [128, CIDX], i16, kind="ExternalInput")
    SVAL = nc.dram_tensor("SVAL", [TG, 128, WDST], fp8, kind="ExternalInput")
    DINV = nc.dram_tensor("DINV", [128, NG], f32, kind="ExternalInput")
    W1T = nc.dram_tensor("W1T", [F_IN, HID], f32, kind="ExternalInput")
    B1T = nc.dram_tensor("B1T", [128, HID], f32, kind="ExternalInput")
    W2T = nc.dram_tensor("W2T", [HID, OUT_D], f32, kind="ExternalInput")
    B2T = nc.dram_tensor("B2T", [128, OUT_D], f32, kind="ExternalInput")
    OUTE = nc.dram_tensor("OUTE", [SH, OUT_D], f32, kind="ExternalOutput")

    t_local = nc.dram_tensor("t_local", [SH, F_IN], bf16)
    t_full = nc.dram_tensor("t_full", [NTOT, F_IN], bf16, addr_space="Shared")

    with tile.TileContext(nc) as tc:
        with (
            tc.tile_pool(name="resident", bufs=1) as rpool,
            tc.tile_pool(name="msg", bufs=8) as mpool,
            tc.tile_pool(name="sv", bufs=3) as spool,
            tc.tile_pool(name="post", bufs=4) as ppool,
            tc.tile_pool(name="psum", bufs=4, space="PSUM") as psum_pool,
            tc.tile_pool(name="psum2", bufs=4, space="PSUM") as psum_pool2,
        ):
            # resident tiles
            idx_t = rpool.tile([128, CIDX], i16)
            nc.sync.dma_start(idx_t[:], IDX[:])
            dinv_t = rpool.tile([128, NG], f32)
            nc.sync.dma_start(dinv_t[:], DINV[:])
            w1_t = rpool.tile([F_IN, HID], f32)
            nc.sync.dma_start(w1_t[:], W1T[:])
            b1_t = rpool.tile([128, HID], f32)
            nc.sync.dma_start(b1_t[:], B1T[:])
            w2_t = rpool.tile([HID, OUT_D], f32)
            nc.sync.dma_start(w2_t[:], W2T[:])
            b2_t = rpool.tile([128, OUT_D], f32)
            nc.sync.dma_start(b2_t[:], B2T[:])

            agg1 = rpool.tile([128, SH], f32)
            agg2 = rpool.tile([128, SH], f32)

            def scatter_layer(table, agg):
                """agg[:, dst] += sum over edges of table[src] (feature-major)."""
                goff = 0
                coff = 0
                for w in range(NWIN):
                    wlen = min(WDST, NPC - w * WDST)
                    cells = n_cell[w]
                    gw = sum((n + 127) // 128 for n in cells)
                    s_t = spool.tile([128, GMAX * NSEC, WDST], fp8, tag="sval")
                    nc.sync.dma_start(
                        s_t[:, :gw, :],
                        SVAL[goff:goff + gw].rearrange("g p w -> p g w"),
                    )
                    acc = psum_pool.tile([128, WDST], f32, tag="acc")
                    total_mm = gw
                    mm = 0
                    ci = coff
                    for s in range(NSEC):
                        n = cells[s]
                        g = (n + 127) // 128
                        ncol = (n + 15) // 16
                        msg = mpool.tile([128, GMAX, F_IN], bf16, tag="msg")
                        nc.gpsimd.dma_gather(
                            msg[:, :g, :],
                            table[s * SEC:(s + 1) * SEC, :],
                            idx_t[:, ci:ci + ncol],
                            n,
                            n,
                            F_IN,
                        )
                        for j in range(g):
                            ns = min(128, n - j * 128)
                            nc.tensor.matmul(
                                acc[:, :],
                                msg[:ns, j, :],
                                s_t[:ns, mm, :],
                                start=(mm == 0),
                                stop=(mm == total_mm - 1),
                            )
                            mm += 1
                        ci += ncol
                    nc.vector.tensor_copy(
                        agg[:, w * WDST:w * WDST + wlen], acc[:, :wlen]
                    )
                    goff += gw
                    coff = ci
                # zero the pad columns (nodes 12500..12543)
                nc.vector.memset(agg[:, NPC:SH], 0.0)

            # ---------------- layer 1 ----------------
            scatter_layer(T1, agg1)

            for g in range(NG):
                ph = psum_pool2.tile([128, HID], f32, tag="wout")
                nc.tensor.matmul(
                    ph[:], agg1[:, g * 128:(g + 1) * 128], w1_t[:],
                    start=True, stop=True,
                )
                tmp = ppool.tile([128, HID], f32, tag="tmp")
                nc.vector.tensor_scalar(
                    out=tmp[:], in0=ph[:], scalar1=dinv_t[:, g:g + 1],
                    scalar2=None, op0=mybir.AluOpType.mult,
                )
                nc.vector.tensor_tensor(
                    out=tmp[:], in0=tmp[:], in1=b1_t[:], op=mybir.AluOpType.add
                )
                t_out = ppool.tile([128, F_IN], bf16, tag="tout")
                nc.vector.memset(t_out[:, HID:], 0.0)
                nc.vector.tensor_scalar(
                    out=t_out[:, :HID], in0=tmp[:], scalar1=0.0,
                    scalar2=dinv_t[:, g:g + 1], op0=mybir.AluOpType.max,
                    op1=mybir.AluOpType.mult,
                )
                nc.sync.dma_start(t_local[g * 128:(g + 1) * 128, :], t_out[:])

            nc.gpsimd.collective_compute(
                "AllGather",
                mybir.AluOpType.bypass,
                replica_groups=[list(range(NCORES))],
                ins=[t_local[:]],
                outs=[t_full[:]],
            )

            # ---------------- layer 2 ----------------
            scatter_layer(t_full, agg2)

            for g in range(NG):
                po = psum_pool2.tile([128, OUT_D], f32, tag="wout")
                nc.tensor.matmul(
                    po[:], agg2[:HID, g * 128:(g + 1) * 128], w2_t[:],
                    start=True, stop=True,
                )
                ot = ppool.tile([128, OUT_D], f32, tag="ot")
                nc.vector.tensor_scalar(
                    out=ot[:], in0=po[:], scalar1=dinv_t[:, g:g + 1],
                    scalar2=None, op0=mybir.AluOpType.mult,
                )
                nc.vector.tensor_tensor(
                    out=ot[:], in0=ot[:], in1=b2_t[:], op=mybir.AluOpType.add
                )
                nc.sync.dma_start(OUTE[g * 128:(g + 1) * 128, :], ot[:])

    nc.compile()
    return nc


# ============================ entry point ============================

def prepare(x, edge_index, W1, b1, W2, b2):
    """Host prep + program build + per-core input maps."""
    x = np.asarray(x, dtype=np.float32)
    W1 = np.asarray(W1, dtype=np.float32)
    b1 = np.asarray(b1, dtype=np.float32)
    W2 = np.asarray(W2, dtype=np.float32)
    b2 = np.asarray(b2, dtype=np.float32)

    sched, idx_all, sval_all, dinv_gt, dinv = _host_prep(edge_index)

    key = (sched["CIDX"], sched["TG"], tuple(tuple(r) for r in sched["n_cell"]))
    if key in _CACHE:
        nc = _CACHE[key]
    else:
        nc = build_program(sched)
        _CACHE[key] = nc

    # table T1 = bf16(dinv * x) in shard layout, pad rows zero
    xs = x * dinv[:, None]
    T1 = np.zeros((NTOT, F_IN), dtype=ml_dtypes.bfloat16)
    for c in range(NCORES):
        T1[c * SH:c * SH + NPC] = xs[c * NPC:(c + 1) * NPC].astype(ml_dtypes.bfloat16)

    b1_tile = np.tile(b1[None, :], (128, 1)).astype(np.float32)
    b2_tile = np.tile(b2[None, :], (128, 1)).astype(np.float32)

    in_maps = []
    for c in range(NCORES):
        in_maps.append({
            "T1": T1,
            "IDX": np.ascontiguousarray(idx_all[c]),
            "SVAL": np.ascontiguousarray(sval_all[c]),
            "DINV": np.ascontiguousarray(dinv_gt[c]),
            "W1T": W1,
            "B1T": b1_tile,
            "W2T": W2,
            "B2T": b2_tile,
        })
    return nc, in_maps


def kernel(x, edge_index, W1, b1, W2, b2):
    from concourse.bass_utils import run_bass_kernel_spmd

    nc, in_maps = prepare(x, edge_index, W1, b1, W2, b2)
    r = run_bass_kernel_spmd(nc, in_maps, core_ids=list(range(NCORES)))
    out = np.empty((N, OUT_D), dtype=np.float32)
    for c in range(NCORES):
        out[c * NPC:(c + 1) * NPC] = r.results[c]["OUTE"][:NPC]
    return out


# revision 9
# speedup vs baseline: 1.2332x; 1.2332x over previous
"""Trainium2 Bass kernel for 2-layer GCN (nn_GCN_22866405884174).

Strategy (8 NeuronCores, dst-node sharding):
  out = A @ relu((A @ x) @ W1 + b1) @ W2 + b2   with A = D^-1/2 (Adj+I) D^-1/2
  (linear layers commute with aggregation, so each layer is: gather table
  rows by edge src + scatter-add by edge dst, then a small dense matmul).

  - Nodes sharded contiguously: core c owns dst nodes [c*12500, (c+1)*12500).
  - Host does graph preprocessing only (degrees, D^-1/2, edge sort, one-hot
    scatter blocks) per the sharding hint.
  - Layer tables are bf16 pre-scaled by dinv[src]; dinv[dst] applied
    post-aggregation on device. Scatter matrices S are exact one-hot fp8.
  - Gather: SWDGE dma_gather, one big instruction per (8-window block,
    table section) to amortize Q7 descriptor-gen + completion latency.
  - Scatter-add: PE matmul msg[slots,feat]^T @ S[slots,128dst] accumulated
    into a full PSUM bank [128, 512] per block (bank-wide has_written clear
    on the first matmul; later matmuls overwrite-or-accumulate per element).
  - Between layers: AllGather of the bf16 hidden table across the 8 cores.
"""

import numpy as np
import ml_dtypes

# ---------------- problem constants (hardcoded per contract) ----------------
N = 100000
E = 1600000
F_IN = 128
HID = 64
OUT_D = 10

NCORES = 8
NPC = N // NCORES           # 12500 nodes per core
SH = 12544                  # padded shard rows (98 * 128)
NTOT = SH * NCORES          # 100352
SEC = 25088                 # table section rows (2 shards, < int16 range)
NSEC = 4
WDST = 64                   # dst window width
NWIN = (NPC + WDST - 1) // WDST   # 196 (last window = 20 dst)
WB = 8                      # windows per block
NBLK = (NWIN + WB - 1) // WB      # 25 (last block = 4 windows)
BCOLS = WB * WDST           # 512 psum cols per block
NG = SH // 128              # 98 node groups per shard
SENTINEL = 12500            # zero pad row (same local idx in every section)

_CACHE = {}


# ============================ host preprocessing ============================

def _host_prep(edge_index):
    src = np.asarray(edge_index[0]).astype(np.int64)
    dst = np.asarray(edge_index[1]).astype(np.int64)
    loops = np.arange(N, dtype=np.int64)
    src = np.concatenate([src, loops])
    dst = np.concatenate([dst, loops])
    deg = np.bincount(dst, minlength=N).astype(np.float32)
    dinv = 1.0 / np.sqrt(deg)

    srow = (src // NPC) * SH + (src % NPC)
    core = dst // NPC
    dloc = dst % NPC
    win = dloc // WDST
    sec = srow // SEC

    cellid = (core * NWIN + win) * NSEC + sec
    counts = np.bincount(cellid, minlength=NCORES * NWIN * NSEC).reshape(NCORES, NWIN, NSEC)
    n_cell = counts.max(axis=0)
    n_cell = np.maximum(((n_cell + 15) // 16) * 16, 128)   # 16-aligned, >= 128

    # ---- block schedule (shared across cores) ----
    # per (block, section): total slots, group psum bases, idx col offsets
    blocks = []
    CIDX = 0
    TG = 0
    for b in range(NBLK):
        wlo, whi = b * WB, min(NWIN, (b + 1) * WB)
        bsec = []
        for s in range(NSEC):
            cells = [int(n_cell[w, s]) for w in range(wlo, whi)]
            nbs = sum(cells)
            ngrp = (nbs + 127) // 128
            # group -> block-local window of its first slot -> psum base col
            bounds = np.cumsum([0] + cells)
            bases = []
            for j in range(ngrp):
                wi = int(np.searchsorted(bounds, j * 128, side="right") - 1)
                bases.append(min(wi * WDST, BCOLS - 128))
            bsec.append({
                "cells": cells, "nbs": nbs, "ngrp": ngrp,
                "bases": bases, "ci": CIDX, "gi": TG,
            })
            CIDX += nbs // 16
            TG += ngrp
        blocks.append(bsec)

    sort_key = (sec + NSEC * (win + NWIN * core))
    order = np.lexsort((dloc, sort_key))
    srow_s = srow[order]
    dloc_s = dloc[order]
    key_s = sort_key[order]

    idx_all = np.zeros((NCORES, 128, CIDX), dtype=np.int16)
    sval_all = np.zeros((NCORES, TG, 128, 128), dtype=ml_dtypes.float8_e4m3)
    dinv_gt = np.zeros((NCORES, 128, NG), dtype=np.float32)

    cw_starts = np.searchsorted(key_s, np.arange(NCORES * NWIN * NSEC + 1))
    for c in range(NCORES):
        s_g = []
        s_p = []
        s_d = []
        for b in range(NBLK):
            wlo, whi = b * WB, min(NWIN, (b + 1) * WB)
            for s in range(NSEC):
                info = blocks[b][s]
                nbs = info["nbs"]
                loc = np.full(nbs, SENTINEL, dtype=np.int64)
                dcol = np.full(nbs, -1, dtype=np.int64)
                off = 0
                for wi, w in enumerate(range(wlo, whi)):
                    cid = (c * NWIN + w) * NSEC + s
                    a, e = cw_starts[cid], cw_starts[cid + 1]
                    cnt = e - a
                    loc[off:off + cnt] = srow_s[a:e] - s * SEC
                    dcol[off:off + cnt] = dloc_s[a:e] - b * BCOLS
                    off += info["cells"][wi]
                # idx (16-wrapped, replicated to 8 q7 groups)
                ci = info["ci"]
                ncol = nbs // 16
                idx_all[c, :, ci:ci + ncol] = np.tile(
                    loc.astype(np.int16).reshape(ncol, 16).T, (8, 1))
                # S one-hot coords for real slots
                t = np.arange(nbs)
                real = dcol >= 0
                g = t // 128
                scol = dcol - np.array(info["bases"], dtype=np.int64)[g]
                if real.any():
                    assert scol[real].min() >= 0 and scol[real].max() < 128
                s_g.append(info["gi"] + g[real])
                s_p.append(t[real] % 128)
                s_d.append(scol[real])
        sval_all[c, np.concatenate(s_g), np.concatenate(s_p),
                 np.concatenate(s_d)] = ml_dtypes.float8_e4m3(1.0)

        dpad = np.zeros(SH, dtype=np.float32)
        dpad[:NPC] = dinv[c * NPC:(c + 1) * NPC]
        dinv_gt[c] = dpad.reshape(NG, 128).T

    sched = {"blocks": blocks, "CIDX": CIDX, "TG": TG}
    return sched, idx_all, sval_all, dinv_gt, dinv


# ============================ device program ============================

def build_program(sched, use_prepare=True):
    import concourse.bass as bass
    import concourse.bacc as bacc
    import concourse.tile as tile
    import concourse.mybir as mybir

    CIDX = sched["CIDX"]
    TG = sched["TG"]
    blocks = sched["blocks"]
    GBS_MAX = max(info["ngrp"] for bsec in blocks for info in bsec)
    GBLK_MAX = max(sum(info["ngrp"] for info in bsec) for bsec in blocks)

    nc = bacc.Bacc(None, target_bir_lowering=False, debug=False)
    f32 = mybir.dt.float32
    bf16 = mybir.dt.bfloat16
    fp8 = mybir.dt.float8e4
    i16 = mybir.dt.int16

    T1 = nc.dram_tensor("T1", [NTOT, F_IN], bf16, kind="ExternalInput")
    IDX = nc.dram_tensor("IDX", [128, CIDX], i16, kind="ExternalInput")
    SVAL = nc.dram_tensor("SVAL", [TG, 128, 128], fp8, kind="ExternalInput")
    DINV = nc.dram_tensor("DINV", [128, NG], f32, kind="ExternalInput")
    W1T = nc.dram_tensor("W1T", [F_IN, HID], f32, kind="ExternalInput")
    B1T = nc.dram_tensor("B1T", [128, HID], f32, kind="ExternalInput")
    W2T = nc.dram_tensor("W2T", [HID, OUT_D], f32, kind="ExternalInput")
    B2T = nc.dram_tensor("B2T", [128, OUT_D], f32, kind="ExternalInput")
    OUTE = nc.dram_tensor("OUTE", [SH, OUT_D], f32, kind="ExternalOutput")

    t_local = nc.dram_tensor("t_local", [SH, F_IN], bf16)
    t_full = nc.dram_tensor("t_full", [NTOT, F_IN], bf16, addr_space="Shared")

    dma_sem = nc.alloc_semaphore("gsem") if use_prepare else None

    with tile.TileContext(nc) as tc:
        with (
            tc.tile_pool(name="resident", bufs=1) as rpool,
            tc.tile_pool(name="msg", bufs=6) as mpool,
            tc.tile_pool(name="sv", bufs=3) as spool,
            tc.tile_pool(name="post", bufs=4) as ppool,
            tc.tile_pool(name="psum", bufs=3, space="PSUM") as psum_pool,
            tc.tile_pool(name="psum2", bufs=4, space="PSUM") as psum_pool2,
        ):
            idx_t = rpool.tile([128, CIDX], i16)
            nc.sync.dma_start(idx_t[:], IDX[:])
            dinv_t = rpool.tile([128, NG], f32)
            nc.sync.dma_start(dinv_t[:], DINV[:])
            w1_t = rpool.tile([F_IN, HID], f32)
            nc.sync.dma_start(w1_t[:], W1T[:])
            b1_t = rpool.tile([128, HID], f32)
            nc.sync.dma_start(b1_t[:], B1T[:])
            w2_t = rpool.tile([HID, OUT_D], f32)
            nc.sync.dma_start(w2_t[:], W2T[:])
            b2_t = rpool.tile([128, OUT_D], f32)
            nc.sync.dma_start(b2_t[:], B2T[:])

            agg1 = rpool.tile([128, SH], f32)
            agg2 = rpool.tile([HID, SH], f32)

            def scatter_layer(table, agg, nfeat):
                for b in range(NBLK):
                    blo = b * BCOLS
                    blen = min(BCOLS, NPC - blo)
                    bsec = blocks[b]
                    gblk = sum(info["ngrp"] for info in bsec)
                    g0 = bsec[0]["gi"]
                    s_t = spool.tile([128, GBLK_MAX, 128], fp8, tag="sval")
                    nc.sync.dma_start(
                        s_t[:, :gblk, :],
                        SVAL[g0:g0 + gblk].rearrange("g p w -> p g w"),
                    )
                    acc = psum_pool.tile([128, BCOLS], f32, tag="acc")
                    mm = 0
                    msgs = []
                    for s in range(NSEC):
                        info = bsec[s]
                        nbs = info["nbs"]
                        ngrp = info["ngrp"]
                        msg = mpool.tile([128, GBS_MAX, F_IN], bf16, tag="msg")
                        gi = nc.gpsimd.dma_gather(
                            msg[:, :ngrp, :],
                            table[s * SEC:(s + 1) * SEC, :],
                            idx_t[:, info["ci"]:info["ci"] + nbs // 16],
                            nbs,
                            nbs,
                            F_IN,
                            prepare_only=use_prepare,
                            sem=dma_sem,
                            single_packet=False,
                        )
                        if use_prepare:
                            nc.gpsimd.trigger_dma(count=None)
                        msgs.append(msg)
                    for s in range(NSEC):
                        info = bsec[s]
                        nbs = info["nbs"]
                        msg = msgs[s]
                        for j in range(info["ngrp"]):
                            ns = min(128, nbs - j * 128)
                            base = info["bases"][j]
                            nc.tensor.matmul(
                                acc[:nfeat, base:base + 128],
                                msg[:ns, j, :nfeat],
                                s_t[:ns, (info["gi"] - g0) + j, :],
                                start=(mm == 0),
                                stop=(mm == gblk - 1),
                            )
                            mm += 1
                    nc.vector.tensor_copy(
                        agg[:, blo:blo + blen], acc[:agg.shape[0], :blen]
                    )
                nc.vector.memset(agg[:, NPC:SH], 0.0)

            # ---------------- layer 1 ----------------
            scatter_layer(T1, agg1, F_IN)

            for g in range(NG):
                ph = psum_pool2.tile([128, HID], f32, tag="wout")
                nc.tensor.matmul(
                    ph[:], agg1[:, g * 128:(g + 1) * 128], w1_t[:],
                    start=True, stop=True,
                )
                tmp = ppool.tile([128, HID], f32, tag="tmp")
                nc.vector.tensor_scalar(
                    out=tmp[:], in0=ph[:], scalar1=dinv_t[:, g:g + 1],
                    scalar2=None, op0=mybir.AluOpType.mult,
                )
                nc.vector.tensor_tensor(
                    out=tmp[:], in0=tmp[:], in1=b1_t[:], op=mybir.AluOpType.add
                )
                t_out = ppool.tile([128, F_IN], bf16, tag="tout")
                nc.vector.memset(t_out[:, HID:], 0.0)
                nc.vector.tensor_scalar(
                    out=t_out[:, :HID], in0=tmp[:], scalar1=0.0,
                    scalar2=dinv_t[:, g:g + 1], op0=mybir.AluOpType.max,
                    op1=mybir.AluOpType.mult,
                )
                nc.sync.dma_start(t_local[g * 128:(g + 1) * 128, :], t_out[:])

            nc.gpsimd.collective_compute(
                "AllGather",
                mybir.AluOpType.bypass,
                replica_groups=[list(range(NCORES))],
                ins=[t_local[:]],
                outs=[t_full[:]],
            )

            # ---------------- layer 2 ----------------
            scatter_layer(t_full, agg2, F_IN)

            for g in range(NG):
                po = psum_pool2.tile([128, OUT_D], f32, tag="wout")
                nc.tensor.matmul(
                    po[:], agg2[:, g * 128:(g + 1) * 128], w2_t[:],
                    start=True, stop=True,
                )
                ot = ppool.tile([128, OUT_D], f32, tag="ot")
                nc.vector.tensor_scalar(
                    out=ot[:], in0=po[:], scalar1=dinv_t[:, g:g + 1],
                    scalar2=None, op0=mybir.AluOpType.mult,
                )
                nc.vector.tensor_tensor(
                    out=ot[:], in0=ot[:], in1=b2_t[:], op=mybir.AluOpType.add
                )
                nc.sync.dma_start(OUTE[g * 128:(g + 1) * 128, :], ot[:])

    nc.compile()
    return nc


# ============================ entry point ============================

def prepare(x, edge_index, W1, b1, W2, b2):
    x = np.asarray(x, dtype=np.float32)
    W1 = np.asarray(W1, dtype=np.float32)
    b1 = np.asarray(b1, dtype=np.float32)
    W2 = np.asarray(W2, dtype=np.float32)
    b2 = np.asarray(b2, dtype=np.float32)

    sched, idx_all, sval_all, dinv_gt, dinv = _host_prep(edge_index)

    import os
    use_prepare = os.environ.get("GCN_NO_PREP", "0") != "1"
    key = ("v2", sched["CIDX"], sched["TG"], use_prepare)
    if key in _CACHE:
        nc = _CACHE[key]
    else:
        nc = build_program(sched, use_prepare=use_prepare)
        _CACHE[key] = nc

    xs = x * dinv[:, None]
    T1 = np.zeros((NTOT, F_IN), dtype=ml_dtypes.bfloat16)
    for c in range(NCORES):
        T1[c * SH:c * SH + NPC] = xs[c * NPC:(c + 1) * NPC].astype(ml_dtypes.bfloat16)

    b1_tile = np.tile(b1[None, :], (128, 1)).astype(np.float32)
    b2_tile = np.tile(b2[None, :], (128, 1)).astype(np.float32)

    in_maps = []
    for c in range(NCORES):
        in_maps.append({
            "T1": T1,
            "IDX": np.ascontiguousarray(idx_all[c]),
            "SVAL": np.ascontiguousarray(sval_all[c]),
            "DINV": np.ascontiguousarray(dinv_gt[c]),
            "W1T": W1,
            "B1T": b1_tile,
            "W2T": W2,
            "B2T": b2_tile,
        })
    return nc, in_maps


def kernel(x, edge_index, W1, b1, W2, b2):
    from concourse.bass_utils import run_bass_kernel_spmd

    nc, in_maps = prepare(x, edge_index, W1, b1, W2, b2)
    r = run_bass_kernel_spmd(nc, in_maps, core_ids=list(range(NCORES)))
    out = np.empty((N, OUT_D), dtype=np.float32)
    for c in range(NCORES):
        out[c * NPC:(c + 1) * NPC] = r.results[c]["OUTE"][:NPC]
    return out
